# revision 6
# baseline (speedup 1.0000x reference)
"""Trainium2 kernel for nn_AdaptiveMetaLearnerV2 (P=1e6, H=20, 8 cores).

Algorithmic reduction: every coordinate of x passes through the SAME tiny
network independently, so the module is a pair of scalar maps
(x_out[p], act[p]) = (F_out(x[p]), F_act(x[p])), qt = mean(act).
With b1 == 0 the first LayerNorm collapses analytically:

    LN1(x*w1) = tau * (w1-mean(w1))/std(w1) * g1 + be1,
    tau = sw*x / (sw*|x| + EPS),  sw = std(w1, ddof=1),  tau in [-1, 1].

So F = G(tau).  G has eps-kinks at tau=0 from the inner LayerNorms, but the
tau-map sends all |x| >= ~1e-3 to |tau| >= 0.9: 99.9% of elements live in
|tau| in [0.9, 1] where G is glass-smooth.  We fit even/odd parts of G there
in a recentered zeta = (tau^2 - c0)/hw (degrees 2-3 reach ~1e-6), evaluate
on-device with fused scalar_tensor_tensor Horner steps, and exactly patch
the ~800 elements with |x| < x_cut on the host (float64 reference math).

Device per-core program (shard = 125k elements as [128, 977], zero-padded):
    ACT: sw|x| -> Ln(.+eps) -> Exp(-.) == r = 1/(sw|x|+eps)
    DVE: tau = (x*sw)*r ; zeta ; two even + two odd Horner chains ; combines
    ACT: Sigmoid with fused per-partition accum for qt
Host: gather x_out, patch, qt = (sum - pad - patch corrections)/1e6.
"""

import numpy as np

H = 20
EPS = 1e-5
P_TOTAL = 1_000_000
N_CORES = 8
SHARD = P_TOTAL // N_CORES          # 125000
FREE = 977                          # 128*977 = 125056 >= SHARD
NPAD = 128 * FREE - SHARD           # 56 zero-pad elements per core

TAU_STAR = 0.9
FIT_TOL = 1.5e-6
MAX_DEG = 10
N_CHUNKS = 2

_CACHE = {}


# ----------------------------------------------------------------- host math
def _sigmoid(z):
    return 1.0 / (1.0 + np.exp(-np.clip(z, -60, 60)))


def _ln1d(v, g, b):
    m = v.mean(axis=-1, keepdims=True)
    s = v.std(axis=-1, ddof=1, keepdims=True)
    return (v - m) / (s + EPS) * g + b


def _cell(x, wi, gi, bi, bh, gc, bc):
    # hx = cx = 0: LN(hx@wh.T) == bh (constant); forget gate multiplies cx==0
    gates = _ln1d(x @ wi.T, gi, bi) + bh
    i, _f, o, g = np.split(gates, 4, axis=-1)
    c = _sigmoid(i) * np.tanh(g)
    return _sigmoid(o) * np.tanh(_ln1d(c, gc, bc))


def _net_of_tau(tau, d):
    """Exact (G_out, s_a) in float64 as functions of tau (includes all eps)."""
    w1 = d['w1'].ravel()
    A = (w1 - w1.mean()) / w1.std(ddof=1)
    xt = np.tanh(tau[:, None] * (A * d['g1'])[None, :] + d['be1'][None, :])
    for l in (0, 1):
        xt = _cell(xt, d[f'wi{l}'], d[f'gi{l}'], d[f'bi{l}'],
                   d[f'bh{l}'], d[f'gc{l}'], d[f'bc{l}'])
    out = xt @ d['wo'].T + d['bo']
    sa = xt @ d['wa'].T + d['ba']
    return out.ravel(), sa.ravel()


def _fit1(zeta, target, tol):
    for deg in range(1, MAX_DEG + 1):
        c = np.polynomial.chebyshev.chebfit(zeta, target, deg)
        p = np.polynomial.chebyshev.cheb2poly(c)
        if np.abs(np.polynomial.polynomial.polyval(zeta, p) - target).max() < tol:
            return p
    return p


class _Fit:
    """Even/odd polynomial model of (G_out, s_a) on |tau| in [TAU_STAR, 1]."""

    def __init__(self, d):
        self.sw = d['w1'].ravel().std(ddof=1)
        self.c0 = (TAU_STAR ** 2 + 1.0) / 2.0
        self.hw = (1.0 - TAU_STAR ** 2) / 2.0
        t = np.sqrt(np.linspace(TAU_STAR ** 2, 1.0, 20001))
        gp_out, gp_sa = _net_of_tau(t, d)
        gm_out, gm_sa = _net_of_tau(-t, d)
        zeta = (t * t - self.c0) / self.hw
        self.eo = _fit1(zeta, (gp_out + gm_out) / 2, FIT_TOL)         # even, out
        self.oo = _fit1(zeta, (gp_out - gm_out) / 2 / t, FIT_TOL)     # odd, out
        self.ea = _fit1(zeta, (gp_sa + gm_sa) / 2, FIT_TOL)           # even, sa
        self.oa = _fit1(zeta, (gp_sa - gm_sa) / 2 / t, FIT_TOL)       # odd, sa

    def tau_of(self, x):
        return self.sw * x / (np.abs(x) * self.sw + EPS)

    def eval_poly(self, tau):
        """Float64 replica of the device computation (for corrections)."""
        P = np.polynomial.polynomial.polyval
        zeta = (tau * tau - self.c0) / self.hw
        out = P(zeta, self.eo) + tau * P(zeta, self.oo)
        sa = P(zeta, self.ea) + tau * P(zeta, self.oa)
        return out, _sigmoid(sa)


# -------------------------------------------------------------- bass builder
def _emit_chain(nc, pool, W, zeta, p, tag, AF, OP, f32):
    """w = P(zeta) - p[0]  (constant term left for the combine step)."""
    d = len(p) - 1
    w = pool.tile([128, W], f32, tag=tag, name=tag)
    nc.scalar.activation(w, zeta, AF.Copy, scale=float(p[d]))
    for k in range(d - 1, 0, -1):
        nc.vector.scalar_tensor_tensor(
            w, w, float(p[k]), zeta, op0=OP.add, op1=OP.mult)
    return w


def _build(fit, n_chunks):
    import concourse.bacc as bacc
    import concourse.mybir as mybir
    from concourse.tile import TileContext

    f32 = mybir.dt.float32
    AF = mybir.ActivationFunctionType
    OP = mybir.AluOpType

    nc = bacc.Bacc()
    x_d = nc.dram_tensor("x", [128, FREE], f32, kind="ExternalInput")
    out_d = nc.dram_tensor("out", [128, FREE], f32, kind="ExternalOutput")
    q_d = nc.dram_tensor("qsum", [128, 1], f32, kind="ExternalOutput")

    bounds = np.linspace(0, FREE, n_chunks + 1).astype(int)
    sw = float(fit.sw)

    with TileContext(nc) as tc:
        with tc.tile_pool(name="pool", bufs=2) as pool, \
             tc.tile_pool(name="qpool", bufs=1) as qpool:
            qparts = qpool.tile([128, n_chunks], f32)
            eps_t = qpool.tile([128, 1], f32)
            one_ap = nc.const_aps.tensor(1.0, (128, 1), f32)
            nc.scalar.activation(eps_t, one_ap, AF.Copy, scale=0.0,
                                 bias=float(EPS))
            for ci in range(n_chunks):
                a, b = int(bounds[ci]), int(bounds[ci + 1])
                W = b - a
                x = pool.tile([128, W], f32, tag="x")
                nc.sync.dma_start(x, x_d[:, a:b])
                r = pool.tile([128, W], f32, tag="r")
                nc.scalar.activation(r, x, AF.Abs, scale=sw)
                nc.scalar.activation(r, r, AF.Ln, bias=eps_t[:, 0:1])
                nc.scalar.activation(r, r, AF.Exp, scale=-1.0)
                tau = pool.tile([128, W], f32, tag="tau")
                nc.vector.scalar_tensor_tensor(
                    tau, x, sw, r, op0=OP.mult, op1=OP.mult)
                sq = pool.tile([128, W], f32, tag="sq")
                nc.vector.scalar_tensor_tensor(
                    sq, tau, float(1.0 / fit.hw), tau, op0=OP.mult, op1=OP.mult)
                zeta = pool.tile([128, W], f32, tag="zeta")
                nc.scalar.activation(zeta, sq, AF.Copy,
                                     bias=float(-fit.c0 / fit.hw))
                # ---- x_out = Eo(zeta) + tau*Oo(zeta)
                wE = _emit_chain(nc, pool, W, zeta, fit.eo, "wE", AF, OP, f32)
                wO = _emit_chain(nc, pool, W, zeta, fit.oo, "wO", AF, OP, f32)
                nc.vector.scalar_tensor_tensor(
                    wO, wO, float(fit.oo[0]), tau, op0=OP.add, op1=OP.mult)
                yo = pool.tile([128, W], f32, tag="yo")
                nc.vector.scalar_tensor_tensor(
                    yo, wE, float(fit.eo[0]), wO, op0=OP.add, op1=OP.add)
                nc.sync.dma_start(out_d[:, a:b], yo)
                # ---- s_a -> sigmoid -> fused per-partition accumulate
                vE = _emit_chain(nc, pool, W, zeta, fit.ea, "vE", AF, OP, f32)
                vO = _emit_chain(nc, pool, W, zeta, fit.oa, "vO", AF, OP, f32)
                nc.vector.scalar_tensor_tensor(
                    vO, vO, float(fit.oa[0]), tau, op0=OP.add, op1=OP.mult)
                pre = pool.tile([128, W], f32, tag="pre")
                nc.vector.scalar_tensor_tensor(
                    pre, vE, float(fit.ea[0]), vO, op0=OP.add, op1=OP.add)
                sig = pool.tile([128, W], f32, tag="sig")
                nc.scalar.activation(sig, pre, AF.Sigmoid,
                                     accum_out=qparts[:, ci:ci + 1])
            q = qpool.tile([128, 1], f32)
            nc.vector.tensor_reduce(q, qparts[:, :], axis=mybir.AxisListType.X,
                                    op=OP.add)
            nc.sync.dma_start(q_d[:, :], q)
    nc.finalize()
    return nc


def _get_compiled(inputs, n_chunks=N_CHUNKS):
    d = {k: np.asarray(v, dtype=np.float64) for k, v in inputs.items()
         if k != 'x'}
    key = (hash(tuple(sorted((k, v.tobytes()) for k, v in d.items()))),
           n_chunks)
    if key in _CACHE:
        return _CACHE[key]
    assert np.abs(d['b1']).max() == 0.0, "b1 != 0 breaks the LN1 reduction"
    fit = _Fit(d)
    nc = _build(fit, n_chunks)
    entry = (nc, fit, d)
    _CACHE[key] = entry
    return entry


def make_in_maps(x):
    shards = np.asarray(x, np.float32).ravel().reshape(N_CORES, SHARD)
    in_maps = []
    for i in range(N_CORES):
        xp = np.zeros(128 * FREE, dtype=np.float32)
        xp[:SHARD] = shards[i]
        in_maps.append({"x": xp.reshape(128, FREE)})
    return in_maps


def postprocess(results, x, fit, d):
    x = np.asarray(x, np.float64).ravel()
    x_out = np.concatenate(
        [np.asarray(r["out"], np.float32).reshape(-1)[:SHARD]
         for r in results]).astype(np.float64)
    qtot = float(sum(np.asarray(r["qsum"], np.float64).sum()
                     for r in results))
    # pad elements (x=0 -> tau=0) contribute the device poly value at tau=0
    _, sig0 = fit.eval_poly(np.zeros(1))
    qtot -= N_CORES * NPAD * float(sig0[0])
    # exact host patch for |x| below the fit domain
    x_cut = TAU_STAR * EPS / (fit.sw * (1.0 - TAU_STAR))
    idx = np.where(np.abs(x) < x_cut)[0]
    if idx.size:
        tau_p = fit.tau_of(x[idx])
        out_dev, sig_dev = fit.eval_poly(tau_p)       # what the device wrote
        out_ex, sa_ex = _net_of_tau(tau_p, d)
        sig_ex = _sigmoid(sa_ex)
        x_out[idx] = out_ex
        qtot += float((sig_ex - sig_dev).sum())
    qt = np.array([qtot / P_TOTAL], dtype=np.float32)
    return x_out.reshape(P_TOTAL, 1).astype(np.float32), qt


# ------------------------------------------------------------------- kernel
def kernel(**inputs):
    from concourse.bass_utils import run_bass_kernel_spmd

    nc, fit, d = _get_compiled(inputs)
    x = np.asarray(inputs['x'], dtype=np.float32).ravel()
    assert x.shape == (P_TOTAL,)
    res = run_bass_kernel_spmd(nc, make_in_maps(x),
                               core_ids=list(range(N_CORES)))
    return postprocess(res.results, x, fit, d)


# revision 10
# speedup vs baseline: 1.0774x; 1.0774x over previous
"""Trainium2 kernel for nn_AdaptiveMetaLearnerV2 (P=1e6, H=20, 8 cores).

Algorithmic reduction: every coordinate of x passes through the SAME tiny
network independently, so the module is a pair of scalar maps
(x_out[p], act[p]) = (F_out(x[p]), F_act(x[p])), qt = mean(act).
With b1 == 0 the first LayerNorm collapses analytically:

    LN1(x*w1) = tau * (w1-mean(w1))/std(w1) * g1 + be1,
    tau = sw*x / (sw*|x| + EPS),  sw = std(w1, ddof=1),  tau in [-1, 1].

So F = G(tau).  G has eps-kinks at tau=0 from the inner LayerNorms, but the
tau-map sends all |x| >= ~2e-3 to |tau| >= 0.95: 99.8% of elements live in
|tau| in [0.95, 1] where G is glass-smooth.  We fit even/odd parts of G_out
and of sigmoid(s_a) there as degree-2 polynomials in the recentered
zeta = (tau^2-c0)/hw (~5e-7 max err), evaluate on-device, and exactly patch
the ~1700 elements with |x| < x_cut on the host (float64 reference math).

Per-core device program (raw bass, no Tile; shard = [128, 977], zero-pad):
  SYNC : dma x_c in | dma yo_c out | dma qsum | final sem reset
  ACT  : Abs -> Ln(.+eps) -> Exp(-.) == r = 1/(sw|x|+eps);
         final x_out affine (Copy scale/bias) -- one table set total
  DVE  : tau=(x*sw)*r ; zeta ; fused x_out Horner (top coeffs deferred into
         the combine scalar and the ACT affine)
  GPSIM: eps memset; the act-function Horner with fused accum_out partial
         sums (its deferred scale/constant are applied to the sum on host)
Engines self-serialize internally (ACT ops don't overlap; DVE drains per
op), so only cross-engine edges need sems: standalone wait_ge instructions.
qt = (oa2 * device_sum + ea0*N - pad - patch corrections) / 1e6 on host.
"""

import contextlib

import numpy as np

H = 20
EPS = 1e-5
P_TOTAL = 1_000_000
N_CORES = 8
SHARD = P_TOTAL // N_CORES          # 125000
FREE = 977                          # 128*977 = 125056 >= SHARD
NPAD = 128 * FREE - SHARD           # 56 zero-pad elements per core

TAU_STAR = 0.95
FIT_TOL = 5e-6
CHUNK_BOUNDS = [0, 326, 651, 977]
GP_ACT = False                      # Pool rejects TensorScalarPtr in ISA

_CACHE = {}


# ----------------------------------------------------------------- host math
def _sigmoid(z):
    return 1.0 / (1.0 + np.exp(-np.clip(z, -60, 60)))


def _ln1d(v, g, b):
    m = v.mean(axis=-1, keepdims=True)
    s = v.std(axis=-1, ddof=1, keepdims=True)
    return (v - m) / (s + EPS) * g + b


def _cell(x, wi, gi, bi, bh, gc, bc):
    # hx = cx = 0: LN(hx@wh.T) == bh (constant); forget gate multiplies cx==0
    gates = _ln1d(x @ wi.T, gi, bi) + bh
    i, _f, o, g = np.split(gates, 4, axis=-1)
    c = _sigmoid(i) * np.tanh(g)
    return _sigmoid(o) * np.tanh(_ln1d(c, gc, bc))


def _net_of_tau(tau, d):
    """Exact (G_out, s_a) in float64 as functions of tau (includes all eps)."""
    w1 = d['w1'].ravel()
    A = (w1 - w1.mean()) / w1.std(ddof=1)
    xt = np.tanh(tau[:, None] * (A * d['g1'])[None, :] + d['be1'][None, :])
    for l in (0, 1):
        xt = _cell(xt, d[f'wi{l}'], d[f'gi{l}'], d[f'bi{l}'],
                   d[f'bh{l}'], d[f'gc{l}'], d[f'bc{l}'])
    out = xt @ d['wo'].T + d['bo']
    sa = xt @ d['wa'].T + d['ba']
    return out.ravel(), sa.ravel()


def _fit2(zeta, target):
    c = np.polynomial.chebyshev.chebfit(zeta, target, 2)
    p = np.polynomial.chebyshev.cheb2poly(c)
    err = np.abs(np.polynomial.polynomial.polyval(zeta, p) - target).max()
    assert err < FIT_TOL, f"degree-2 fit err {err:.2e} exceeds {FIT_TOL}"
    assert abs(p[2]) > 1e-12
    return p


class _Fit:
    """Degree-2 even/odd model on |tau| in [TAU_STAR, 1]:
    G = E(zeta) + tau*O(zeta), zeta = (tau^2-c0)/hw."""

    def __init__(self, d):
        self.sw = d['w1'].ravel().std(ddof=1)
        self.c0 = (TAU_STAR ** 2 + 1.0) / 2.0
        self.hw = (1.0 - TAU_STAR ** 2) / 2.0
        t = np.sqrt(np.linspace(TAU_STAR ** 2, 1.0, 20001))
        gp_out, gp_sa = _net_of_tau(t, d)
        gm_out, gm_sa = _net_of_tau(-t, d)
        gp_act, gm_act = _sigmoid(gp_sa), _sigmoid(gm_sa)
        zeta = (t * t - self.c0) / self.hw
        self.eo = _fit2(zeta, (gp_out + gm_out) / 2)           # even, out
        self.oo = _fit2(zeta, (gp_out - gm_out) / 2 / t)       # odd, out
        self.ea = _fit2(zeta, (gp_act + gm_act) / 2)           # even, act
        self.oa = _fit2(zeta, (gp_act - gm_act) / 2 / t)       # odd, act

    def tau_of(self, x):
        return self.sw * x / (np.abs(x) * self.sw + EPS)

    def eval_poly(self, tau):
        """Float64 replica of the device computation (for corrections)."""
        P = np.polynomial.polynomial.polyval
        zeta = (tau * tau - self.c0) / self.hw
        out = P(zeta, self.eo) + tau * P(zeta, self.oo)
        act = P(zeta, self.ea) + tau * P(zeta, self.oa)
        return out, act


# -------------------------------------------------------- raw bass builder
def _build(fit):
    import concourse.bass as bass
    import concourse.mybir as mybir

    f32 = mybir.dt.float32
    AF = mybir.ActivationFunctionType
    OP = mybir.AluOpType

    nc = bass.Bass()
    x_d = nc.dram_tensor("x", [128, FREE], f32, kind="ExternalInput")
    out_d = nc.dram_tensor("out", [128, FREE], f32, kind="ExternalOutput")
    q_d = nc.dram_tensor("qsum", [128, 1], f32, kind="ExternalOutput")

    sw = float(fit.sw)
    inv_hw = float(1.0 / fit.hw)
    zoff = float(-fit.c0 / fit.hw)
    eo, oo, ea, oa = fit.eo, fit.oo, fit.ea, fit.oa
    bounds = CHUNK_BOUNDS
    NCH = len(bounds) - 1
    widths = [bounds[i + 1] - bounds[i] for i in range(NCH)]

    ctx = contextlib.ExitStack()
    sem = lambda name: ctx.enter_context(nc.semaphore(name))
    sb = lambda name, W: ctx.enter_context(nc.sbuf_tensor(name, [128, W], f32))

    with ctx:
        s_eps = sem("s_eps")
        s_in = sem("s_in")
        s_r = sem("s_r")
        s_zeta = sem("s_zeta")
        s_comb = sem("s_comb")
        s_gq = sem("s_gq")
        s_yo = sem("s_yo")
        s_q = sem("s_q")
        s_out = sem("s_out")
        all_sems = [s_eps, s_in, s_r, s_zeta, s_comb, s_gq, s_yo, s_q, s_out]

        eps_t = sb("eps_t", 1)
        qparts = sb("qparts", NCH)
        qred = sb("qred", 1)
        X, T, R, TAU, ZETA = [], [], [], [], []
        WE, WO, VE, VO, YP, YO = [], [], [], [], [], []
        for c, W in enumerate(widths):
            X.append(sb(f"x{c}", W)); T.append(sb(f"t{c}", W))
            R.append(sb(f"r{c}", W)); TAU.append(sb(f"tau{c}", W))
            ZETA.append(sb(f"zeta{c}", W))
            WE.append(sb(f"wE{c}", W)); WO.append(sb(f"wO{c}", W))
            VE.append(sb(f"vE{c}", W)); VO.append(sb(f"vO{c}", W))
            YP.append(sb(f"yp{c}", W)); YO.append(sb(f"yo{c}", W))
        qdump = sb("qdump", max(widths))

        with nc.Block() as block:

            @block.sync
            def _(sync):
                for c in range(NCH):
                    a, b = bounds[c], bounds[c + 1]
                    sync.dma_start(X[c][:, :], x_d[:, a:b]).then_inc(s_in, 16)
                for c in range(NCH):
                    a, b = bounds[c], bounds[c + 1]
                    sync.wait_ge(s_yo, c + 1)
                    sync.dma_start(out_d[:, a:b], YO[c][:, :]).then_inc(s_out, 16)
                sync.wait_ge(s_q, 1)
                sync.dma_start(q_d[:, :], qred[:, :]).then_inc(s_out, 16)
                sync.wait_ge(s_out, 16 * (NCH + 1))
                for s in all_sems:
                    sync.sem_clear(s)

            @block.scalar
            def _(scalar):
                scalar.wait_ge(s_eps, 1)
                for c in range(NCH):
                    # r = exp(-ln(sw|x| + eps)) = 1/(sw|x|+eps)
                    scalar.wait_ge(s_in, 16 * (c + 1))
                    nc.scalar.activation(T[c][:, :], X[c][:, :], AF.Abs,
                                         scale=sw)
                    nc.scalar.activation(R[c][:, :], T[c][:, :], AF.Ln,
                                         bias=eps_t[:, 0:1])
                    nc.scalar.activation(R[c][:, :], R[c][:, :], AF.Exp,
                                         scale=-1.0).then_inc(s_r, 1)
                for c in range(NCH):
                    # x_out = oo2 * yp + eo0
                    scalar.wait_ge(s_comb, c + 1)
                    nc.scalar.activation(YO[c][:, :], YP[c][:, :], AF.Copy,
                                         scale=float(oo[2]), bias=float(eo[0])
                                         ).then_inc(s_yo, 1)

            @block.vector
            def _(vector):
                stt = nc.vector.scalar_tensor_tensor
                ts = nc.vector.tensor_scalar
                for c in range(NCH):
                    Z = ZETA[c][:, :]
                    vector.wait_ge(s_r, c + 1)
                    stt(TAU[c][:, :], X[c][:, :], sw, R[c][:, :],
                        op0=OP.mult, op1=OP.mult)
                    stt(Z, TAU[c][:, :], inv_hw, TAU[c][:, :],
                        op0=OP.mult, op1=OP.mult)
                    ts(Z, Z, zoff, None, OP.add).then_inc(s_zeta, 1)
                    # x_out chain: yp = (eo2/oo2)*wE + wO, wE=(z+eo1/eo2)z,
                    # wO = ((z+oo1/oo2)z + oo0/oo2)*tau
                    stt(WE[c][:, :], Z, float(eo[1] / eo[2]), Z,
                        op0=OP.add, op1=OP.mult)
                    stt(WO[c][:, :], Z, float(oo[1] / oo[2]), Z,
                        op0=OP.add, op1=OP.mult)
                    stt(WO[c][:, :], WO[c][:, :], float(oo[0] / oo[2]),
                        TAU[c][:, :], op0=OP.add, op1=OP.mult)
                    stt(YP[c][:, :], WE[c][:, :], float(eo[2] / oo[2]),
                        WO[c][:, :], op0=OP.mult, op1=OP.add
                        ).then_inc(s_comb, 1)
                    if not GP_ACT:
                        _emit_act_chain(nc.vector, nc, c, widths, ZETA, TAU,
                                        VE, VO, qdump, qparts, ea, oa, OP,
                                        s_gq)
                vector.wait_ge(s_gq, NCH)
                nc.vector.tensor_reduce(
                    qred[:, :], qparts[:, :], axis=mybir.AxisListType.X,
                    op=OP.add).then_inc(s_q, 1)

            @block.gpsimd
            def _(gpsimd):
                gpsimd.memset(eps_t[:, :], float(EPS)).then_inc(s_eps, 1)
                if GP_ACT:
                    for c in range(NCH):
                        gpsimd.wait_ge(s_zeta, c + 1)
                        _emit_act_chain(nc.gpsimd, nc, c, widths, ZETA, TAU,
                                        VE, VO, qdump, qparts, ea, oa, OP,
                                        s_gq)

    nc.finalize()
    return nc


def _emit_act_chain(eng, nc, c, widths, ZETA, TAU, VE, VO, qdump, qparts,
                    ea, oa, OP, s_gq):
    """act/oa2 (sans ea0 term): vE=(z+ea1/ea2)z; vO=((z+oa1/oa2)z+oa0/oa2)*tau;
    v = (ea2/oa2)*vE + vO, partial sums via accum_out."""
    Z = ZETA[c][:, :]
    stt = eng.scalar_tensor_tensor
    stt(VE[c][:, :], Z, float(ea[1] / ea[2]), Z, op0=OP.add, op1=OP.mult)
    stt(VO[c][:, :], Z, float(oa[1] / oa[2]), Z, op0=OP.add, op1=OP.mult)
    stt(VO[c][:, :], VO[c][:, :], float(oa[0] / oa[2]), TAU[c][:, :],
        op0=OP.add, op1=OP.mult)
    stt(qdump[:, :widths[c]], VE[c][:, :], float(ea[2] / oa[2]),
        VO[c][:, :], op0=OP.mult, op1=OP.add,
        accum_out=qparts[:, c:c + 1]).then_inc(s_gq, 1)


def _get_compiled(inputs):
    d = {k: np.asarray(v, dtype=np.float64) for k, v in inputs.items()
         if k != 'x'}
    key = hash(tuple(sorted((k, v.tobytes()) for k, v in d.items())))
    if key in _CACHE:
        return _CACHE[key]
    assert np.abs(d['b1']).max() == 0.0, "b1 != 0 breaks the LN1 reduction"
    fit = _Fit(d)
    nc = _build(fit)
    entry = (nc, fit, d)
    _CACHE[key] = entry
    return entry


def make_in_maps(x):
    shards = np.asarray(x, np.float32).ravel().reshape(N_CORES, SHARD)
    in_maps = []
    for i in range(N_CORES):
        xp = np.zeros(128 * FREE, dtype=np.float32)
        xp[:SHARD] = shards[i]
        in_maps.append({"x": xp.reshape(128, FREE)})
    return in_maps


def postprocess(results, x, fit, d):
    x = np.asarray(x, np.float64).ravel()
    x_out = np.concatenate(
        [np.asarray(r["out"], np.float32).reshape(-1)[:SHARD]
         for r in results]).astype(np.float64)
    # device qsum holds sum of (act - ea0)/oa2 over ALL 128*FREE elements
    vsum = float(sum(np.asarray(r["qsum"], np.float64).sum()
                     for r in results))
    qtot = float(fit.oa[2]) * vsum + float(fit.ea[0]) * (N_CORES * 128 * FREE)
    # pad elements (x=0 -> tau=0) contribute the device act value at tau=0
    _, act0 = fit.eval_poly(np.zeros(1))
    qtot -= N_CORES * NPAD * float(act0[0])
    # exact host patch for |x| below the fit domain
    x_cut = TAU_STAR * EPS / (fit.sw * (1.0 - TAU_STAR))
    idx = np.where(np.abs(x) < x_cut)[0]
    if idx.size:
        tau_p = fit.tau_of(x[idx])
        out_dev, act_dev = fit.eval_poly(tau_p)       # what the device wrote
        out_ex, sa_ex = _net_of_tau(tau_p, d)
        x_out[idx] = out_ex
        qtot += float((_sigmoid(sa_ex) - act_dev).sum())
    qt = np.array([qtot / P_TOTAL], dtype=np.float32)
    return x_out.reshape(P_TOTAL, 1).astype(np.float32), qt


# ------------------------------------------------------------------- kernel
def kernel(**inputs):
    from concourse.bass_utils import run_bass_kernel_spmd

    nc, fit, d = _get_compiled(inputs)
    x = np.asarray(inputs['x'], dtype=np.float32).ravel()
    assert x.shape == (P_TOTAL,)
    res = run_bass_kernel_spmd(nc, make_in_maps(x),
                               core_ids=list(range(N_CORES)))
    return postprocess(res.results, x, fit, d)


# revision 13
# speedup vs baseline: 1.3078x; 1.2138x over previous
"""Trainium2 kernel for nn_AdaptiveMetaLearnerV2 (P=1e6, H=20, 8 cores).

Algorithmic reduction: every coordinate of x passes through the SAME tiny
network independently, so the module is a pair of scalar maps
(x_out[p], act[p]) = (F_out(x[p]), F_act(x[p])), qt = mean(act).
With b1 == 0 the first LayerNorm collapses analytically:

    LN1(x*w1) = tau * (w1-mean(w1))/std(w1) * g1 + be1,
    tau = sw*x / (sw*|x| + EPS),  sw = std(w1, ddof=1),  tau in [-1, 1].

So F = G(tau).  G has eps-kinks at tau=0 from the inner LayerNorms, but the
tau-map sends all |x| >= ~2e-3 to |tau| >= 0.95: 99.8% of elements live in
|tau| in [0.95, 1] where G is glass-smooth.  We fit even/odd parts of G_out
and of sigmoid(s_a) there as degree-2 polynomials in the recentered
zeta = (tau^2-c0)/hw (~5e-7 max err), evaluate on-device, and exactly patch
the ~1700 elements with |x| < x_cut on the host (float64 reference math).

Per-core device program (raw bass, no Tile; shard = [128, 977], zero-pad):
  SYNC : dma x_c in | dma yo_c out | dma qsum | final sem reset
  ACT  : Abs -> Ln(.+eps) -> Exp(-.) == r = 1/(sw|x|+eps);
         final x_out affine (Copy scale/bias) -- one table set total
  DVE  : tau=(x*sw)*r ; zeta ; fused x_out Horner (top coeffs deferred into
         the combine scalar and the ACT affine)
  GPSIM: eps memset; the act-function Horner with fused accum_out partial
         sums (its deferred scale/constant are applied to the sum on host)
Engines self-serialize internally (ACT ops don't overlap; DVE drains per
op), so only cross-engine edges need sems: standalone wait_ge instructions.
qt = (oa2 * device_sum + ea0*N - pad - patch corrections) / 1e6 on host.
"""

import contextlib

import numpy as np

H = 20
EPS = 1e-5
P_TOTAL = 1_000_000
N_CORES = 8
SHARD = P_TOTAL // N_CORES          # 125000
FREE = 978                          # 128*978 = 125184 >= SHARD
NPAD = 128 * FREE - SHARD           # 184 zero-pad elements per core

TAU_STAR = 0.95
FIT_TOL = 5e-6
NCH = 2
CW = FREE // NCH                    # 489 columns per chunk

_CACHE = {}


# ----------------------------------------------------------------- host math
def _sigmoid(z):
    return 1.0 / (1.0 + np.exp(-np.clip(z, -60, 60)))


def _ln1d(v, g, b):
    m = v.mean(axis=-1, keepdims=True)
    s = v.std(axis=-1, ddof=1, keepdims=True)
    return (v - m) / (s + EPS) * g + b


def _cell(x, wi, gi, bi, bh, gc, bc):
    # hx = cx = 0: LN(hx@wh.T) == bh (constant); forget gate multiplies cx==0
    gates = _ln1d(x @ wi.T, gi, bi) + bh
    i, _f, o, g = np.split(gates, 4, axis=-1)
    c = _sigmoid(i) * np.tanh(g)
    return _sigmoid(o) * np.tanh(_ln1d(c, gc, bc))


def _net_of_tau(tau, d):
    """Exact (G_out, s_a) in float64 as functions of tau (includes all eps)."""
    w1 = d['w1'].ravel()
    A = (w1 - w1.mean()) / w1.std(ddof=1)
    xt = np.tanh(tau[:, None] * (A * d['g1'])[None, :] + d['be1'][None, :])
    for l in (0, 1):
        xt = _cell(xt, d[f'wi{l}'], d[f'gi{l}'], d[f'bi{l}'],
                   d[f'bh{l}'], d[f'gc{l}'], d[f'bc{l}'])
    out = xt @ d['wo'].T + d['bo']
    sa = xt @ d['wa'].T + d['ba']
    return out.ravel(), sa.ravel()


def _fit2(zeta, target):
    c = np.polynomial.chebyshev.chebfit(zeta, target, 2)
    p = np.polynomial.chebyshev.cheb2poly(c)
    err = np.abs(np.polynomial.polynomial.polyval(zeta, p) - target).max()
    assert err < FIT_TOL, f"degree-2 fit err {err:.2e} exceeds {FIT_TOL}"
    assert abs(p[2]) > 1e-12
    return p


class _Fit:
    """Degree-2 even/odd model on |tau| in [TAU_STAR, 1]:
    G = E(zeta) + tau*O(zeta), zeta = (tau^2-c0)/hw."""

    def __init__(self, d):
        self.sw = d['w1'].ravel().std(ddof=1)
        self.c0 = (TAU_STAR ** 2 + 1.0) / 2.0
        self.hw = (1.0 - TAU_STAR ** 2) / 2.0
        t = np.sqrt(np.linspace(TAU_STAR ** 2, 1.0, 20001))
        gp_out, gp_sa = _net_of_tau(t, d)
        gm_out, gm_sa = _net_of_tau(-t, d)
        gp_act, gm_act = _sigmoid(gp_sa), _sigmoid(gm_sa)
        zeta = (t * t - self.c0) / self.hw
        self.eo = _fit2(zeta, (gp_out + gm_out) / 2)           # even, out
        self.oo = _fit2(zeta, (gp_out - gm_out) / 2 / t)       # odd, out
        self.ea = _fit2(zeta, (gp_act + gm_act) / 2)           # even, act
        self.oa = _fit2(zeta, (gp_act - gm_act) / 2 / t)       # odd, act

    def tau_of(self, x):
        return self.sw * x / (np.abs(x) * self.sw + EPS)

    def eval_poly(self, tau):
        """Float64 replica of the device computation (for corrections)."""
        P = np.polynomial.polynomial.polyval
        zeta = (tau * tau - self.c0) / self.hw
        out = P(zeta, self.eo) + tau * P(zeta, self.oo)
        act = P(zeta, self.ea) + tau * P(zeta, self.oa)
        return out, act


# -------------------------------------------------------- raw bass builder
def _build(fit):
    import concourse.bass as bass
    import concourse.mybir as mybir

    f32 = mybir.dt.float32
    AF = mybir.ActivationFunctionType
    OP = mybir.AluOpType

    from unittest import mock

    # no Tile: orderings are carried by explicit semaphores, so the heavy
    # full-drain barriers (init + Block-end, ~13us of EVENT_SEMAPHORE churn
    # and engine drains per execution) are replaced by the cheap
    # sequencer-level sem-only barrier, which still terminates every
    # engine's stream.
    def _cheap_barrier(self, **kw):
        for inst in self._sem_only_all_engine_barrier_insts(
                f"aeb{self.next_id()}"):
            self.engines[inst.engine].add_instruction(inst)

    barrier_patch = mock.patch.object(bass.Bass, "all_engine_barrier",
                                      _cheap_barrier)
    barrier_patch.start()
    try:
        nc = _build_inner(bass, mybir, f32, AF, OP, fit)
    finally:
        barrier_patch.stop()
    return nc


def _build_inner(bass, mybir, f32, AF, OP, fit):
    nc = bass.Bass()
    # chunk-contiguous layout: one dense 250KB transfer per chunk
    x_d = nc.dram_tensor("x", [NCH, 128, CW], f32, kind="ExternalInput")
    out_d = nc.dram_tensor("out", [NCH, 128, CW], f32, kind="ExternalOutput")
    q_d = nc.dram_tensor("qsum", [128, 1], f32, kind="ExternalOutput")

    sw = float(fit.sw)
    inv_hw = float(1.0 / fit.hw)
    zoff = float(-fit.c0 / fit.hw)
    eo, oo, ea, oa = fit.eo, fit.oo, fit.ea, fit.oa
    widths = [CW] * NCH

    ctx = contextlib.ExitStack()
    sem = lambda name: ctx.enter_context(nc.semaphore(name))
    sb = lambda name, W: ctx.enter_context(nc.sbuf_tensor(name, [128, W], f32))

    with ctx:
        s_eps = sem("s_eps")
        s_in = sem("s_in")
        s_r = sem("s_r")
        s_zeta = sem("s_zeta")
        s_comb = sem("s_comb")
        s_gq = sem("s_gq")
        s_yo = sem("s_yo")
        s_q = sem("s_q")
        s_out = sem("s_out")
        all_sems = [s_eps, s_in, s_r, s_zeta, s_comb, s_gq, s_yo, s_q, s_out]

        eps_t = sb("eps_t", 1)
        scratch = sb("scratch", 1)
        qparts = sb("qparts", NCH)
        qred = sb("qred", 1)
        X, T, R, TAU, ZETA = [], [], [], [], []
        WE, WO, VE, VO, YP, YO = [], [], [], [], [], []
        for c, W in enumerate(widths):
            X.append(sb(f"x{c}", W)); T.append(sb(f"t{c}", W))
            R.append(sb(f"r{c}", W)); TAU.append(sb(f"tau{c}", W))
            ZETA.append(sb(f"zeta{c}", W))
            WE.append(sb(f"wE{c}", W)); WO.append(sb(f"wO{c}", W))
            VE.append(sb(f"vE{c}", W)); VO.append(sb(f"vO{c}", W))
            YP.append(sb(f"yp{c}", W)); YO.append(sb(f"yo{c}", W))
        qdump = sb("qdump", max(widths))

        with nc.Block() as block:

            @block.sync
            def _(sync):
                for c in range(NCH):
                    sync.dma_start(X[c][:, :], x_d[c, :, :]).then_inc(s_in, 16)
                for c in range(NCH):
                    sync.wait_ge(s_yo, c + 1)
                    sync.dma_start(out_d[c, :, :], YO[c][:, :]).then_inc(s_out, 16)
                sync.wait_ge(s_q, 1)
                sync.dma_start(q_d[:, :], qred[:, :]).then_inc(s_out, 16)
                sync.wait_ge(s_out, 16 * (NCH + 1))
                for s in all_sems:
                    sync.sem_clear(s)

            @block.scalar
            def _(scalar):
                # dummy op before any wait: anchors the one ACT_TABLE_LOAD
                # (ln/exp set) at t~0, overlapped with the input DMA
                nc.scalar.activation(scratch[:, 0:1], eps_t[:, 0:1], AF.Ln)
                scalar.wait_ge(s_eps, 1)
                for c in range(NCH):
                    # r = exp(-ln(sw|x| + eps)) = 1/(sw|x|+eps)
                    scalar.wait_ge(s_in, 16 * (c + 1))
                    nc.scalar.activation(T[c][:, :], X[c][:, :], AF.Abs,
                                         scale=sw)
                    nc.scalar.activation(R[c][:, :], T[c][:, :], AF.Ln,
                                         bias=eps_t[:, 0:1])
                    nc.scalar.activation(R[c][:, :], R[c][:, :], AF.Exp,
                                         scale=-1.0).then_inc(s_r, 1)
                for c in range(NCH):
                    # x_out = oo2 * yp + eo0
                    scalar.wait_ge(s_comb, c + 1)
                    nc.scalar.activation(YO[c][:, :], YP[c][:, :], AF.Copy,
                                         scale=float(oo[2]), bias=float(eo[0])
                                         ).then_inc(s_yo, 1)

            @block.vector
            def _(vector):
                stt = nc.vector.scalar_tensor_tensor
                ts = nc.vector.tensor_scalar
                for c in range(NCH):
                    Z = ZETA[c][:, :]
                    vector.wait_ge(s_r, c + 1)
                    stt(TAU[c][:, :], X[c][:, :], sw, R[c][:, :],
                        op0=OP.mult, op1=OP.mult)
                    stt(Z, TAU[c][:, :], inv_hw, TAU[c][:, :],
                        op0=OP.mult, op1=OP.mult)
                    ts(Z, Z, zoff, None, OP.add).then_inc(s_zeta, 1)
                    # x_out chain: yp = (eo2/oo2)*wE + wO, wE=(z+eo1/eo2)z,
                    # wO = ((z+oo1/oo2)z + oo0/oo2)*tau
                    stt(WE[c][:, :], Z, float(eo[1] / eo[2]), Z,
                        op0=OP.add, op1=OP.mult)
                    stt(WO[c][:, :], Z, float(oo[1] / oo[2]), Z,
                        op0=OP.add, op1=OP.mult)
                    stt(WO[c][:, :], WO[c][:, :], float(oo[0] / oo[2]),
                        TAU[c][:, :], op0=OP.add, op1=OP.mult)
                    stt(YP[c][:, :], WE[c][:, :], float(eo[2] / oo[2]),
                        WO[c][:, :], op0=OP.mult, op1=OP.add
                        ).then_inc(s_comb, 1)
                    _emit_act_chain(nc.vector, nc, c, widths, ZETA, TAU,
                                    VE, VO, qdump, qparts, ea, oa, OP,
                                    s_gq)
                vector.wait_ge(s_gq, NCH)
                nc.vector.tensor_reduce(
                    qred[:, :], qparts[:, :], axis=mybir.AxisListType.X,
                    op=OP.add).then_inc(s_q, 1)

            @block.gpsimd
            def _(gpsimd):
                gpsimd.memset(eps_t[:, :], float(EPS)).then_inc(s_eps, 1)

    nc.finalize()
    return nc


def _emit_act_chain(eng, nc, c, widths, ZETA, TAU, VE, VO, qdump, qparts,
                    ea, oa, OP, s_gq):
    """act/oa2 (sans ea0 term): vE=(z+ea1/ea2)z; vO=((z+oa1/oa2)z+oa0/oa2)*tau;
    v = (ea2/oa2)*vE + vO, partial sums via accum_out."""
    Z = ZETA[c][:, :]
    stt = eng.scalar_tensor_tensor
    stt(VE[c][:, :], Z, float(ea[1] / ea[2]), Z, op0=OP.add, op1=OP.mult)
    stt(VO[c][:, :], Z, float(oa[1] / oa[2]), Z, op0=OP.add, op1=OP.mult)
    stt(VO[c][:, :], VO[c][:, :], float(oa[0] / oa[2]), TAU[c][:, :],
        op0=OP.add, op1=OP.mult)
    stt(qdump[:, :widths[c]], VE[c][:, :], float(ea[2] / oa[2]),
        VO[c][:, :], op0=OP.mult, op1=OP.add,
        accum_out=qparts[:, c:c + 1]).then_inc(s_gq, 1)


def _get_compiled(inputs):
    d = {k: np.asarray(v, dtype=np.float64) for k, v in inputs.items()
         if k != 'x'}
    key = hash(tuple(sorted((k, v.tobytes()) for k, v in d.items())))
    if key in _CACHE:
        return _CACHE[key]
    assert np.abs(d['b1']).max() == 0.0, "b1 != 0 breaks the LN1 reduction"
    fit = _Fit(d)
    nc = _build(fit)
    entry = (nc, fit, d)
    _CACHE[key] = entry
    return entry


def make_in_maps(x):
    shards = np.asarray(x, np.float32).ravel().reshape(N_CORES, SHARD)
    in_maps = []
    for i in range(N_CORES):
        xp = np.zeros(128 * FREE, dtype=np.float32)
        xp[:SHARD] = shards[i]
        in_maps.append({"x": xp.reshape(128, FREE)})
    return in_maps


def postprocess(results, x, fit, d):
    x = np.asarray(x, np.float64).ravel()
    x_out = np.concatenate(
        [np.asarray(r["out"], np.float32).reshape(-1)[:SHARD]
         for r in results]).astype(np.float64)
    # device qsum holds sum of (act - ea0)/oa2 over ALL 128*FREE elements
    vsum = float(sum(np.asarray(r["qsum"], np.float64).sum()
                     for r in results))
    qtot = float(fit.oa[2]) * vsum + float(fit.ea[0]) * (N_CORES * 128 * FREE)
    # pad elements (x=0 -> tau=0) contribute the device act value at tau=0
    _, act0 = fit.eval_poly(np.zeros(1))
    qtot -= N_CORES * NPAD * float(act0[0])
    # exact host patch for |x| below the fit domain
    x_cut = TAU_STAR * EPS / (fit.sw * (1.0 - TAU_STAR))
    idx = np.where(np.abs(x) < x_cut)[0]
    if idx.size:
        tau_p = fit.tau_of(x[idx])
        out_dev, act_dev = fit.eval_poly(tau_p)       # what the device wrote
        out_ex, sa_ex = _net_of_tau(tau_p, d)
        x_out[idx] = out_ex
        qtot += float((_sigmoid(sa_ex) - act_dev).sum())
    qt = np.array([qtot / P_TOTAL], dtype=np.float32)
    return x_out.reshape(P_TOTAL, 1).astype(np.float32), qt


# ------------------------------------------------------------------- kernel
def kernel(**inputs):
    from concourse.bass_utils import run_bass_kernel_spmd

    nc, fit, d = _get_compiled(inputs)
    x = np.asarray(inputs['x'], dtype=np.float32).ravel()
    assert x.shape == (P_TOTAL,)
    res = run_bass_kernel_spmd(nc, make_in_maps(x),
                               core_ids=list(range(N_CORES)))
    return postprocess(res.results, x, fit, d)


# revision 15
# speedup vs baseline: 1.5369x; 1.1752x over previous
"""Trainium2 kernel for nn_AdaptiveMetaLearnerV2 (P=1e6, H=20, 8 cores).

Algorithmic reduction: every coordinate of x passes through the SAME tiny
network independently, so the module is a pair of scalar maps
(x_out[p], act[p]) = (F_out(x[p]), F_act(x[p])), qt = mean(act).
With b1 == 0 the first LayerNorm collapses analytically:

    LN1(x*w1) = tau * (w1-mean(w1))/std(w1) * g1 + be1,
    tau = sw*x / (sw*|x| + EPS),  sw = std(w1, ddof=1),  tau in [-1, 1].

So F = G(tau).  G has eps-kinks at tau=0 from the inner LayerNorms, but the
tau-map sends all |x| >= ~2e-3 to |tau| >= 0.95: 99.8% of elements live in
|tau| in [0.95, 1] where G is glass-smooth.  We fit even/odd parts of G_out
and of sigmoid(s_a) there as degree-2 polynomials in the recentered
zeta = (tau^2-c0)/hw (~5e-7 max err), evaluate on-device, and exactly patch
the ~1700 elements with |x| < x_cut on the host (float64 reference math).

Per-core device program (raw bass, no Tile; shard = [128, 977], zero-pad):
  SYNC : dma x_c in | dma yo_c out | dma qsum | final sem reset
  ACT  : Abs -> Ln(.+eps) -> Exp(-.) == r = 1/(sw|x|+eps);
         final x_out affine (Copy scale/bias) -- one table set total
  DVE  : tau=(x*sw)*r ; zeta ; fused x_out Horner (top coeffs deferred into
         the combine scalar and the ACT affine)
  GPSIM: eps memset; the act-function Horner with fused accum_out partial
         sums (its deferred scale/constant are applied to the sum on host)
Engines self-serialize internally (ACT ops don't overlap; DVE drains per
op), so only cross-engine edges need sems: standalone wait_ge instructions.
qt = (oa2 * device_sum + ea0*N - pad - patch corrections) / 1e6 on host.
"""

import contextlib

import numpy as np

H = 20
EPS = 1e-5
P_TOTAL = 1_000_000
N_CORES = 8
SHARD = P_TOTAL // N_CORES          # 125000
FREE = 978                          # 128*978 = 125184 >= SHARD
NPAD = 128 * FREE - SHARD           # 184 zero-pad elements per core

TAU_STAR = 0.95
FIT_TOL = 5e-6
NCH = 2
CW = FREE // NCH                    # 489 columns per chunk

_CACHE = {}


# ----------------------------------------------------------------- host math
def _sigmoid(z):
    return 1.0 / (1.0 + np.exp(-np.clip(z, -60, 60)))


def _ln1d(v, g, b):
    m = v.mean(axis=-1, keepdims=True)
    s = v.std(axis=-1, ddof=1, keepdims=True)
    return (v - m) / (s + EPS) * g + b


def _cell(x, wi, gi, bi, bh, gc, bc):
    # hx = cx = 0: LN(hx@wh.T) == bh (constant); forget gate multiplies cx==0
    gates = _ln1d(x @ wi.T, gi, bi) + bh
    i, _f, o, g = np.split(gates, 4, axis=-1)
    c = _sigmoid(i) * np.tanh(g)
    return _sigmoid(o) * np.tanh(_ln1d(c, gc, bc))


def _net_of_tau(tau, d):
    """Exact (G_out, s_a) in float64 as functions of tau (includes all eps)."""
    w1 = d['w1'].ravel()
    A = (w1 - w1.mean()) / w1.std(ddof=1)
    xt = np.tanh(tau[:, None] * (A * d['g1'])[None, :] + d['be1'][None, :])
    for l in (0, 1):
        xt = _cell(xt, d[f'wi{l}'], d[f'gi{l}'], d[f'bi{l}'],
                   d[f'bh{l}'], d[f'gc{l}'], d[f'bc{l}'])
    out = xt @ d['wo'].T + d['bo']
    sa = xt @ d['wa'].T + d['ba']
    return out.ravel(), sa.ravel()


def _fit2(zeta, target):
    c = np.polynomial.chebyshev.chebfit(zeta, target, 2)
    p = np.polynomial.chebyshev.cheb2poly(c)
    err = np.abs(np.polynomial.polynomial.polyval(zeta, p) - target).max()
    assert err < FIT_TOL, f"degree-2 fit err {err:.2e} exceeds {FIT_TOL}"
    assert abs(p[2]) > 1e-12
    return p


class _Fit:
    """Degree-2 even/odd model on |tau| in [TAU_STAR, 1]:
    G = E(zeta) + tau*O(zeta), zeta = (tau^2-c0)/hw."""

    def __init__(self, d):
        self.sw = d['w1'].ravel().std(ddof=1)
        self.c0 = (TAU_STAR ** 2 + 1.0) / 2.0
        self.hw = (1.0 - TAU_STAR ** 2) / 2.0
        t = np.sqrt(np.linspace(TAU_STAR ** 2, 1.0, 20001))
        gp_out, gp_sa = _net_of_tau(t, d)
        gm_out, gm_sa = _net_of_tau(-t, d)
        gp_act, gm_act = _sigmoid(gp_sa), _sigmoid(gm_sa)
        zeta = (t * t - self.c0) / self.hw
        self.eo = _fit2(zeta, (gp_out + gm_out) / 2)           # even, out
        self.oo = _fit2(zeta, (gp_out - gm_out) / 2 / t)       # odd, out
        self.ea = _fit2(zeta, (gp_act + gm_act) / 2)           # even, act
        self.oa = _fit2(zeta, (gp_act - gm_act) / 2 / t)       # odd, act

    def tau_of(self, x):
        return self.sw * x / (np.abs(x) * self.sw + EPS)

    def eval_poly(self, tau):
        """Float64 replica of the device computation (for corrections)."""
        P = np.polynomial.polynomial.polyval
        zeta = (tau * tau - self.c0) / self.hw
        out = P(zeta, self.eo) + tau * P(zeta, self.oo)
        act = P(zeta, self.ea) + tau * P(zeta, self.oa)
        return out, act


# -------------------------------------------------------- raw bass builder
def _build(fit):
    import concourse.bass as bass
    import concourse.mybir as mybir

    f32 = mybir.dt.float32
    AF = mybir.ActivationFunctionType
    OP = mybir.AluOpType

    from unittest import mock

    # no Tile: orderings are carried by explicit semaphores, so the heavy
    # full-drain barriers (init + Block-end, ~13us of EVENT_SEMAPHORE churn
    # and engine drains per execution) are replaced by the cheap
    # sequencer-level sem-only barrier, which still terminates every
    # engine's stream.
    state = {"init_done": False}

    def _cheap_barrier(self, **kw):
        if not state["init_done"]:
            return  # drop the Bass-init barrier: s_eps gates the const reads
        for inst in self._sem_only_all_engine_barrier_insts(
                f"aeb{self.next_id()}"):
            self.engines[inst.engine].add_instruction(inst)

    barrier_patch = mock.patch.object(bass.Bass, "all_engine_barrier",
                                      _cheap_barrier)
    barrier_patch.start()
    try:
        nc = _build_inner(bass, mybir, f32, AF, OP, fit, state)
    finally:
        barrier_patch.stop()
    return nc


def _build_inner(bass, mybir, f32, AF, OP, fit, state):
    nc = bass.Bass()
    state["init_done"] = True
    # chunk-contiguous layout: one dense 250KB transfer per chunk; the last
    # chunk carries one extra column holding the qt partial-sum vector (a
    # separate [128,1] DMA would cost ~6.5us in 4-byte descriptors)
    x_d = nc.dram_tensor("x", [NCH, 128, CW], f32, kind="ExternalInput")
    out_d = nc.dram_tensor("out", [NCH, 128, CW + 1], f32,
                           kind="ExternalOutput")

    sw = float(fit.sw)
    inv_hw = float(1.0 / fit.hw)
    zoff = float(-fit.c0 / fit.hw)
    eo, oo, ea, oa = fit.eo, fit.oo, fit.ea, fit.oa
    widths = [CW] * NCH

    ctx = contextlib.ExitStack()
    sem = lambda name: ctx.enter_context(nc.semaphore(name))
    sb = lambda name, W: ctx.enter_context(nc.sbuf_tensor(name, [128, W], f32))

    with ctx:
        s_eps = sem("s_eps")
        s_in = sem("s_in")
        s_r = sem("s_r")
        s_zeta = sem("s_zeta")
        s_comb = sem("s_comb")
        s_gq = sem("s_gq")
        s_yo = sem("s_yo")
        s_q = sem("s_q")
        s_out = sem("s_out")
        all_sems = [s_eps, s_in, s_r, s_zeta, s_comb, s_gq, s_yo, s_q, s_out]

        eps_t = sb("eps_t", 1)
        scratch = sb("scratch", 1)
        qparts = sb("qparts", NCH)
        qred = sb("qred", 1)
        X, T, R, TAU, ZETA = [], [], [], [], []
        WE, WO, VE, VO, YP, YO = [], [], [], [], [], []
        for c, W in enumerate(widths):
            X.append(sb(f"x{c}", W)); T.append(sb(f"t{c}", W))
            R.append(sb(f"r{c}", W)); TAU.append(sb(f"tau{c}", W))
            ZETA.append(sb(f"zeta{c}", W))
            WE.append(sb(f"wE{c}", W)); WO.append(sb(f"wO{c}", W))
            VE.append(sb(f"vE{c}", W)); VO.append(sb(f"vO{c}", W))
            YP.append(sb(f"yp{c}", W)); YO.append(sb(f"yo{c}", W + 1))
        qdump = sb("qdump", max(widths))

        with nc.Block() as block:

            @block.sync
            def _(sync):
                for c in range(NCH):
                    sync.dma_start(X[c][:, :], x_d[c, :, :]).then_inc(s_in, 16)
                for c in range(NCH):
                    sync.wait_ge(s_yo, c + 1)
                    if c == NCH - 1:
                        sync.wait_ge(s_q, 1)
                    sync.dma_start(out_d[c, :, :], YO[c][:, :]).then_inc(s_out, 16)
                sync.wait_ge(s_out, 16 * NCH)
                for s in all_sems:
                    sync.sem_clear(s)

            @block.scalar
            def _(scalar):
                # dummy op before any wait: anchors the one ACT_TABLE_LOAD
                # (ln/exp set) at t~0, overlapped with the input DMA
                nc.scalar.activation(scratch[:, 0:1], eps_t[:, 0:1], AF.Ln)
                scalar.wait_ge(s_eps, 1)
                for c in range(NCH):
                    # r = exp(-ln(sw|x| + eps)) = 1/(sw|x|+eps)
                    scalar.wait_ge(s_in, 16 * (c + 1))
                    nc.scalar.activation(T[c][:, :], X[c][:, :], AF.Abs,
                                         scale=sw)
                    nc.scalar.activation(R[c][:, :], T[c][:, :], AF.Ln,
                                         bias=eps_t[:, 0:1])
                    nc.scalar.activation(R[c][:, :], R[c][:, :], AF.Exp,
                                         scale=-1.0).then_inc(s_r, 1)
                for c in range(NCH):
                    # x_out = oo2 * yp + eo0
                    scalar.wait_ge(s_comb, c + 1)
                    nc.scalar.activation(YO[c][:, :widths[c]], YP[c][:, :],
                                         AF.Copy, scale=float(oo[2]),
                                         bias=float(eo[0])).then_inc(s_yo, 1)

            @block.vector
            def _(vector):
                stt = nc.vector.scalar_tensor_tensor
                ts = nc.vector.tensor_scalar
                for c in range(NCH):
                    Z = ZETA[c][:, :]
                    vector.wait_ge(s_r, c + 1)
                    stt(TAU[c][:, :], X[c][:, :], sw, R[c][:, :],
                        op0=OP.mult, op1=OP.mult)
                    stt(Z, TAU[c][:, :], inv_hw, TAU[c][:, :],
                        op0=OP.mult, op1=OP.mult)
                    ts(Z, Z, zoff, None, OP.add).then_inc(s_zeta, 1)
                    # x_out chain: yp = (eo2/oo2)*wE + wO, wE=(z+eo1/eo2)z,
                    # wO = ((z+oo1/oo2)z + oo0/oo2)*tau
                    stt(WE[c][:, :], Z, float(eo[1] / eo[2]), Z,
                        op0=OP.add, op1=OP.mult)
                    stt(WO[c][:, :], Z, float(oo[1] / oo[2]), Z,
                        op0=OP.add, op1=OP.mult)
                    stt(WO[c][:, :], WO[c][:, :], float(oo[0] / oo[2]),
                        TAU[c][:, :], op0=OP.add, op1=OP.mult)
                    stt(YP[c][:, :], WE[c][:, :], float(eo[2] / oo[2]),
                        WO[c][:, :], op0=OP.mult, op1=OP.add
                        ).then_inc(s_comb, 1)
                    _emit_act_chain(nc.vector, nc, c, widths, ZETA, TAU,
                                    VE, VO, qdump, qparts, ea, oa, OP,
                                    s_gq)
                vector.wait_ge(s_gq, NCH)
                nc.vector.tensor_reduce(
                    YO[NCH - 1][:, CW:CW + 1], qparts[:, :],
                    axis=mybir.AxisListType.X, op=OP.add).then_inc(s_q, 1)

            @block.gpsimd
            def _(gpsimd):
                gpsimd.memset(eps_t[:, :], float(EPS)).then_inc(s_eps, 1)

    nc.finalize()
    return nc


def _emit_act_chain(eng, nc, c, widths, ZETA, TAU, VE, VO, qdump, qparts,
                    ea, oa, OP, s_gq):
    """act/oa2 (sans ea0 term): vE=(z+ea1/ea2)z; vO=((z+oa1/oa2)z+oa0/oa2)*tau;
    v = (ea2/oa2)*vE + vO, partial sums via accum_out."""
    Z = ZETA[c][:, :]
    stt = eng.scalar_tensor_tensor
    stt(VE[c][:, :], Z, float(ea[1] / ea[2]), Z, op0=OP.add, op1=OP.mult)
    stt(VO[c][:, :], Z, float(oa[1] / oa[2]), Z, op0=OP.add, op1=OP.mult)
    stt(VO[c][:, :], VO[c][:, :], float(oa[0] / oa[2]), TAU[c][:, :],
        op0=OP.add, op1=OP.mult)
    stt(qdump[:, :widths[c]], VE[c][:, :], float(ea[2] / oa[2]),
        VO[c][:, :], op0=OP.mult, op1=OP.add,
        accum_out=qparts[:, c:c + 1]).then_inc(s_gq, 1)


def _get_compiled(inputs):
    d = {k: np.asarray(v, dtype=np.float64) for k, v in inputs.items()
         if k != 'x'}
    key = hash(tuple(sorted((k, v.tobytes()) for k, v in d.items())))
    if key in _CACHE:
        return _CACHE[key]
    assert np.abs(d['b1']).max() == 0.0, "b1 != 0 breaks the LN1 reduction"
    fit = _Fit(d)
    nc = _build(fit)
    entry = (nc, fit, d)
    _CACHE[key] = entry
    return entry


def make_in_maps(x):
    shards = np.asarray(x, np.float32).ravel().reshape(N_CORES, SHARD)
    in_maps = []
    for i in range(N_CORES):
        xp = np.zeros(128 * FREE, dtype=np.float32)
        xp[:SHARD] = shards[i]
        in_maps.append({"x": xp.reshape(128, FREE)})
    return in_maps


def postprocess(results, x, fit, d):
    x = np.asarray(x, np.float64).ravel()
    x_out = np.concatenate(
        [np.asarray(r["out"], np.float32)[:, :, :CW].reshape(-1)[:SHARD]
         for r in results]).astype(np.float64)
    # device qsum vector rides in the last chunk's extra output column;
    # it holds sums of (act - ea0)/oa2 over ALL 128*FREE elements
    vsum = float(sum(np.asarray(r["out"], np.float64)[NCH - 1, :, CW].sum()
                     for r in results))
    qtot = float(fit.oa[2]) * vsum + float(fit.ea[0]) * (N_CORES * 128 * FREE)
    # pad elements (x=0 -> tau=0) contribute the device act value at tau=0
    _, act0 = fit.eval_poly(np.zeros(1))
    qtot -= N_CORES * NPAD * float(act0[0])
    # exact host patch for |x| below the fit domain
    x_cut = TAU_STAR * EPS / (fit.sw * (1.0 - TAU_STAR))
    idx = np.where(np.abs(x) < x_cut)[0]
    if idx.size:
        tau_p = fit.tau_of(x[idx])
        out_dev, act_dev = fit.eval_poly(tau_p)       # what the device wrote
        out_ex, sa_ex = _net_of_tau(tau_p, d)
        x_out[idx] = out_ex
        qtot += float((_sigmoid(sa_ex) - act_dev).sum())
    qt = np.array([qtot / P_TOTAL], dtype=np.float32)
    return x_out.reshape(P_TOTAL, 1).astype(np.float32), qt


# ------------------------------------------------------------------- kernel
def kernel(**inputs):
    from concourse.bass_utils import run_bass_kernel_spmd

    nc, fit, d = _get_compiled(inputs)
    x = np.asarray(inputs['x'], dtype=np.float32).ravel()
    assert x.shape == (P_TOTAL,)
    res = run_bass_kernel_spmd(nc, make_in_maps(x),
                               core_ids=list(range(N_CORES)))
    return postprocess(res.results, x, fit, d)


# revision 17
# speedup vs baseline: 1.5842x; 1.0308x over previous
"""Trainium2 kernel for nn_AdaptiveMetaLearnerV2 (P=1e6, H=20, 8 cores).

Algorithmic reduction: every coordinate of x passes through the SAME tiny
network independently, so the module is a pair of scalar maps
(x_out[p], act[p]) = (F_out(x[p]), F_act(x[p])), qt = mean(act).
With b1 == 0 the first LayerNorm collapses analytically:

    LN1(x*w1) = tau * (w1-mean(w1))/std(w1) * g1 + be1,
    tau = sw*x / (sw*|x| + EPS),  sw = std(w1, ddof=1),  tau in [-1, 1].

So F = G(tau).  G has eps-kinks at tau=0 from the inner LayerNorms, but the
tau-map sends all |x| >= ~2e-3 to |tau| >= 0.95: 99.8% of elements live in
|tau| in [0.95, 1] where G is glass-smooth.  We fit even/odd parts of G_out
and of sigmoid(s_a) there as degree-2 polynomials in the recentered
zeta = (tau^2-c0)/hw (~5e-7 max err), evaluate on-device, and exactly patch
the ~1700 elements with |x| < x_cut on the host (float64 reference math).

Per-core device program (raw bass, no Tile; shard = [128, 977], zero-pad):
  SYNC : dma x_c in | dma yo_c out | dma qsum | final sem reset
  ACT  : Abs -> Ln(.+eps) -> Exp(-.) == r = 1/(sw|x|+eps);
         final x_out affine (Copy scale/bias) -- one table set total
  DVE  : tau=(x*sw)*r ; zeta ; fused x_out Horner (top coeffs deferred into
         the combine scalar and the ACT affine)
  GPSIM: eps memset; the act-function Horner with fused accum_out partial
         sums (its deferred scale/constant are applied to the sum on host)
Engines self-serialize internally (ACT ops don't overlap; DVE drains per
op), so only cross-engine edges need sems: standalone wait_ge instructions.
qt = (oa2 * device_sum + ea0*N - pad - patch corrections) / 1e6 on host.
"""

import contextlib

import numpy as np

H = 20
EPS = 1e-5
P_TOTAL = 1_000_000
N_CORES = 8
SHARD = P_TOTAL // N_CORES          # 125000
FREE = 978                          # 128*978 = 125184 >= SHARD
NPAD = 128 * FREE - SHARD           # 184 zero-pad elements per core

TAU_STAR = 0.95
FIT_TOL = 5e-6
WIDTHS = [200, 600, 178]            # uneven: short lead-in, fast tail drain
NCH = len(WIDTHS)
OFFS = [128 * sum(WIDTHS[:i]) for i in range(NCH)]      # dram elem offsets
OUT_WIDTHS = WIDTHS[:-1] + [WIDTHS[-1] + 1]             # +1 col: qt sums
OUT_OFFS = [128 * sum(OUT_WIDTHS[:i]) for i in range(NCH)]

_CACHE = {}


# ----------------------------------------------------------------- host math
def _sigmoid(z):
    return 1.0 / (1.0 + np.exp(-np.clip(z, -60, 60)))


def _ln1d(v, g, b):
    m = v.mean(axis=-1, keepdims=True)
    s = v.std(axis=-1, ddof=1, keepdims=True)
    return (v - m) / (s + EPS) * g + b


def _cell(x, wi, gi, bi, bh, gc, bc):
    # hx = cx = 0: LN(hx@wh.T) == bh (constant); forget gate multiplies cx==0
    gates = _ln1d(x @ wi.T, gi, bi) + bh
    i, _f, o, g = np.split(gates, 4, axis=-1)
    c = _sigmoid(i) * np.tanh(g)
    return _sigmoid(o) * np.tanh(_ln1d(c, gc, bc))


def _net_of_tau(tau, d):
    """Exact (G_out, s_a) in float64 as functions of tau (includes all eps)."""
    w1 = d['w1'].ravel()
    A = (w1 - w1.mean()) / w1.std(ddof=1)
    xt = np.tanh(tau[:, None] * (A * d['g1'])[None, :] + d['be1'][None, :])
    for l in (0, 1):
        xt = _cell(xt, d[f'wi{l}'], d[f'gi{l}'], d[f'bi{l}'],
                   d[f'bh{l}'], d[f'gc{l}'], d[f'bc{l}'])
    out = xt @ d['wo'].T + d['bo']
    sa = xt @ d['wa'].T + d['ba']
    return out.ravel(), sa.ravel()


def _fit2(zeta, target):
    c = np.polynomial.chebyshev.chebfit(zeta, target, 2)
    p = np.polynomial.chebyshev.cheb2poly(c)
    err = np.abs(np.polynomial.polynomial.polyval(zeta, p) - target).max()
    assert err < FIT_TOL, f"degree-2 fit err {err:.2e} exceeds {FIT_TOL}"
    assert abs(p[2]) > 1e-12
    return p


class _Fit:
    """Degree-2 even/odd model on |tau| in [TAU_STAR, 1]:
    G = E(zeta) + tau*O(zeta), zeta = (tau^2-c0)/hw."""

    def __init__(self, d):
        self.sw = d['w1'].ravel().std(ddof=1)
        self.c0 = (TAU_STAR ** 2 + 1.0) / 2.0
        self.hw = (1.0 - TAU_STAR ** 2) / 2.0
        t = np.sqrt(np.linspace(TAU_STAR ** 2, 1.0, 20001))
        gp_out, gp_sa = _net_of_tau(t, d)
        gm_out, gm_sa = _net_of_tau(-t, d)
        gp_act, gm_act = _sigmoid(gp_sa), _sigmoid(gm_sa)
        zeta = (t * t - self.c0) / self.hw
        self.eo = _fit2(zeta, (gp_out + gm_out) / 2)           # even, out
        self.oo = _fit2(zeta, (gp_out - gm_out) / 2 / t)       # odd, out
        self.ea = _fit2(zeta, (gp_act + gm_act) / 2)           # even, act
        self.oa = _fit2(zeta, (gp_act - gm_act) / 2 / t)       # odd, act

    def tau_of(self, x):
        return self.sw * x / (np.abs(x) * self.sw + EPS)

    def eval_poly(self, tau):
        """Float64 replica of the device computation (for corrections)."""
        P = np.polynomial.polynomial.polyval
        zeta = (tau * tau - self.c0) / self.hw
        out = P(zeta, self.eo) + tau * P(zeta, self.oo)
        act = P(zeta, self.ea) + tau * P(zeta, self.oa)
        return out, act


# -------------------------------------------------------- raw bass builder
def _build(fit):
    import concourse.bass as bass
    import concourse.mybir as mybir

    f32 = mybir.dt.float32
    AF = mybir.ActivationFunctionType
    OP = mybir.AluOpType

    from unittest import mock

    # no Tile: orderings are carried by explicit semaphores, so the heavy
    # full-drain barriers (init + Block-end, ~13us of EVENT_SEMAPHORE churn
    # and engine drains per execution) are replaced by the cheap
    # sequencer-level sem-only barrier, which still terminates every
    # engine's stream.
    state = {"init_done": False}

    def _cheap_barrier(self, **kw):
        if not state["init_done"]:
            return  # drop the Bass-init barrier: s_eps gates the const reads
        for inst in self._sem_only_all_engine_barrier_insts(
                f"aeb{self.next_id()}"):
            self.engines[inst.engine].add_instruction(inst)

    barrier_patch = mock.patch.object(bass.Bass, "all_engine_barrier",
                                      _cheap_barrier)
    barrier_patch.start()
    try:
        nc = _build_inner(bass, mybir, f32, AF, OP, fit, state)
    finally:
        barrier_patch.stop()
    return nc


def _build_inner(bass, mybir, f32, AF, OP, fit, state):
    nc = bass.Bass()
    state["init_done"] = True
    # chunk-contiguous flat layout: one dense transfer per chunk; the last
    # chunk carries one extra column holding the qt partial-sum vector (a
    # separate [128,1] DMA would cost ~6.5us in 4-byte descriptors)
    x_d = nc.dram_tensor("x", [128 * FREE], f32, kind="ExternalInput")
    out_d = nc.dram_tensor("out", [128 * FREE + 128], f32,
                           kind="ExternalOutput")

    def in_ap(c):
        return bass.AP(x_d, OFFS[c], [[WIDTHS[c], 128], [1, WIDTHS[c]]])

    def out_ap(c):
        w = OUT_WIDTHS[c]
        return bass.AP(out_d, OUT_OFFS[c], [[w, 128], [1, w]])

    sw = float(fit.sw)
    inv_hw = float(1.0 / fit.hw)
    zoff = float(-fit.c0 / fit.hw)
    eo, oo, ea, oa = fit.eo, fit.oo, fit.ea, fit.oa
    widths = WIDTHS

    ctx = contextlib.ExitStack()
    sem = lambda name: ctx.enter_context(nc.semaphore(name))
    sb = lambda name, W: ctx.enter_context(nc.sbuf_tensor(name, [128, W], f32))

    with ctx:
        s_eps = sem("s_eps")
        s_in = sem("s_in")
        s_r = sem("s_r")
        s_zeta = sem("s_zeta")
        s_comb = sem("s_comb")
        s_gq = sem("s_gq")
        s_yo = sem("s_yo")
        s_q = sem("s_q")
        s_out = sem("s_out")
        all_sems = [s_eps, s_in, s_r, s_zeta, s_comb, s_gq, s_yo, s_q, s_out]

        eps_t = sb("eps_t", 1)
        scratch = sb("scratch", 1)
        qparts = sb("qparts", NCH)
        qred = sb("qred", 1)
        X, T, R, TAU, ZETA = [], [], [], [], []
        WE, WO, VE, VO, YP, YO = [], [], [], [], [], []
        for c, W in enumerate(widths):
            X.append(sb(f"x{c}", W)); T.append(sb(f"t{c}", W))
            R.append(sb(f"r{c}", W)); TAU.append(sb(f"tau{c}", W))
            ZETA.append(sb(f"zeta{c}", W))
            WE.append(sb(f"wE{c}", W)); WO.append(sb(f"wO{c}", W))
            VE.append(sb(f"vE{c}", W)); VO.append(sb(f"vO{c}", W))
            YP.append(sb(f"yp{c}", W))
            YO.append(sb(f"yo{c}", W + (1 if c == NCH - 1 else 0)))
        qdump = sb("qdump", max(widths))

        with nc.Block() as block:

            @block.sync
            def _(sync):
                for c in range(NCH):
                    sync.dma_start(X[c][:, :], in_ap(c)).then_inc(s_in, 16)
                for c in range(NCH):
                    sync.wait_ge(s_yo, c + 1)
                    if c == NCH - 1:
                        sync.wait_ge(s_q, 1)
                    sync.dma_start(out_ap(c), YO[c][:, :]).then_inc(s_out, 16)
                sync.wait_ge(s_out, 16 * NCH)
                for s in all_sems:
                    sync.sem_clear(s)

            @block.scalar
            def _(scalar):
                # dummy op before any wait: anchors the one ACT_TABLE_LOAD
                # (ln/exp set) at t~0, overlapped with the input DMA
                nc.scalar.activation(scratch[:, 0:1], eps_t[:, 0:1], AF.Ln)
                scalar.wait_ge(s_eps, 1)
                for c in range(NCH):
                    # r = exp(-ln(sw|x| + eps)) = 1/(sw|x|+eps)
                    scalar.wait_ge(s_in, 16 * (c + 1))
                    nc.scalar.activation(T[c][:, :], X[c][:, :], AF.Abs,
                                         scale=sw)
                    nc.scalar.activation(R[c][:, :], T[c][:, :], AF.Ln,
                                         bias=eps_t[:, 0:1])
                    nc.scalar.activation(R[c][:, :], R[c][:, :], AF.Exp,
                                         scale=-1.0).then_inc(s_r, 1)
                for c in range(NCH):
                    # x_out = oo2 * yp + eo0
                    scalar.wait_ge(s_comb, c + 1)
                    nc.scalar.activation(YO[c][:, :widths[c]], YP[c][:, :],
                                         AF.Copy, scale=float(oo[2]),
                                         bias=float(eo[0])).then_inc(s_yo, 1)

            @block.vector
            def _(vector):
                stt = nc.vector.scalar_tensor_tensor
                ts = nc.vector.tensor_scalar
                for c in range(NCH):
                    Z = ZETA[c][:, :]
                    vector.wait_ge(s_r, c + 1)
                    stt(TAU[c][:, :], X[c][:, :], sw, R[c][:, :],
                        op0=OP.mult, op1=OP.mult)
                    stt(Z, TAU[c][:, :], inv_hw, TAU[c][:, :],
                        op0=OP.mult, op1=OP.mult)
                    ts(Z, Z, zoff, None, OP.add).then_inc(s_zeta, 1)
                    # x_out chain: yp = (eo2/oo2)*wE + wO, wE=(z+eo1/eo2)z,
                    # wO = ((z+oo1/oo2)z + oo0/oo2)*tau
                    stt(WE[c][:, :], Z, float(eo[1] / eo[2]), Z,
                        op0=OP.add, op1=OP.mult)
                    stt(WO[c][:, :], Z, float(oo[1] / oo[2]), Z,
                        op0=OP.add, op1=OP.mult)
                    stt(WO[c][:, :], WO[c][:, :], float(oo[0] / oo[2]),
                        TAU[c][:, :], op0=OP.add, op1=OP.mult)
                    stt(YP[c][:, :], WE[c][:, :], float(eo[2] / oo[2]),
                        WO[c][:, :], op0=OP.mult, op1=OP.add
                        ).then_inc(s_comb, 1)
                    _emit_act_chain(nc.vector, nc, c, widths, ZETA, TAU,
                                    VE, VO, qdump, qparts, ea, oa, OP,
                                    s_gq)
                vector.wait_ge(s_gq, NCH)
                W_last = WIDTHS[-1]
                nc.vector.tensor_reduce(
                    YO[NCH - 1][:, W_last:W_last + 1], qparts[:, :],
                    axis=mybir.AxisListType.X, op=OP.add).then_inc(s_q, 1)

            @block.gpsimd
            def _(gpsimd):
                gpsimd.memset(eps_t[:, :], float(EPS)).then_inc(s_eps, 1)

    nc.finalize()
    return nc


def _emit_act_chain(eng, nc, c, widths, ZETA, TAU, VE, VO, qdump, qparts,
                    ea, oa, OP, s_gq):
    """act/oa2 (sans ea0 term): vE=(z+ea1/ea2)z; vO=((z+oa1/oa2)z+oa0/oa2)*tau;
    v = (ea2/oa2)*vE + vO, partial sums via accum_out."""
    Z = ZETA[c][:, :]
    stt = eng.scalar_tensor_tensor
    stt(VE[c][:, :], Z, float(ea[1] / ea[2]), Z, op0=OP.add, op1=OP.mult)
    stt(VO[c][:, :], Z, float(oa[1] / oa[2]), Z, op0=OP.add, op1=OP.mult)
    stt(VO[c][:, :], VO[c][:, :], float(oa[0] / oa[2]), TAU[c][:, :],
        op0=OP.add, op1=OP.mult)
    stt(qdump[:, :widths[c]], VE[c][:, :], float(ea[2] / oa[2]),
        VO[c][:, :], op0=OP.mult, op1=OP.add,
        accum_out=qparts[:, c:c + 1]).then_inc(s_gq, 1)


def _get_compiled(inputs):
    d = {k: np.asarray(v, dtype=np.float64) for k, v in inputs.items()
         if k != 'x'}
    key = hash(tuple(sorted((k, v.tobytes()) for k, v in d.items())))
    if key in _CACHE:
        return _CACHE[key]
    assert np.abs(d['b1']).max() == 0.0, "b1 != 0 breaks the LN1 reduction"
    fit = _Fit(d)
    nc = _build(fit)
    entry = (nc, fit, d)
    _CACHE[key] = entry
    return entry


def make_in_maps(x):
    shards = np.asarray(x, np.float32).ravel().reshape(N_CORES, SHARD)
    in_maps = []
    for i in range(N_CORES):
        xp = np.zeros(128 * FREE, dtype=np.float32)
        xp[:SHARD] = shards[i]
        in_maps.append({"x": xp})
    return in_maps


def postprocess(results, x, fit, d):
    x = np.asarray(x, np.float64).ravel()
    per_core, vsum = [], 0.0
    for r in results:
        flat = np.asarray(r["out"], np.float32).ravel()
        segs = []
        for c in range(NCH):
            blk = flat[OUT_OFFS[c]:OUT_OFFS[c] + 128 * OUT_WIDTHS[c]]
            blk = blk.reshape(128, OUT_WIDTHS[c])
            segs.append(blk[:, :WIDTHS[c]].ravel())
            if c == NCH - 1:
                # qt partial-sum vector rides in the extra output column;
                # sums of (act - ea0)/oa2 over ALL 128*FREE elements
                vsum += float(blk[:, WIDTHS[c]].astype(np.float64).sum())
        per_core.append(np.concatenate(segs)[:SHARD])
    x_out = np.concatenate(per_core).astype(np.float64)
    qtot = float(fit.oa[2]) * vsum + float(fit.ea[0]) * (N_CORES * 128 * FREE)
    # pad elements (x=0 -> tau=0) contribute the device act value at tau=0
    _, act0 = fit.eval_poly(np.zeros(1))
    qtot -= N_CORES * NPAD * float(act0[0])
    # exact host patch for |x| below the fit domain
    x_cut = TAU_STAR * EPS / (fit.sw * (1.0 - TAU_STAR))
    idx = np.where(np.abs(x) < x_cut)[0]
    if idx.size:
        tau_p = fit.tau_of(x[idx])
        out_dev, act_dev = fit.eval_poly(tau_p)       # what the device wrote
        out_ex, sa_ex = _net_of_tau(tau_p, d)
        x_out[idx] = out_ex
        qtot += float((_sigmoid(sa_ex) - act_dev).sum())
    qt = np.array([qtot / P_TOTAL], dtype=np.float32)
    return x_out.reshape(P_TOTAL, 1).astype(np.float32), qt


# ------------------------------------------------------------------- kernel
def kernel(**inputs):
    from concourse.bass_utils import run_bass_kernel_spmd

    nc, fit, d = _get_compiled(inputs)
    x = np.asarray(inputs['x'], dtype=np.float32).ravel()
    assert x.shape == (P_TOTAL,)
    res = run_bass_kernel_spmd(nc, make_in_maps(x),
                               core_ids=list(range(N_CORES)))
    return postprocess(res.results, x, fit, d)


# revision 20
# speedup vs baseline: 1.6384x; 1.0342x over previous
"""Trainium2 kernel for nn_AdaptiveMetaLearnerV2 (P=1e6, H=20, 8 cores).

Algorithmic reduction: every coordinate of x passes through the SAME tiny
network independently, so the module is a pair of scalar maps
(x_out[p], act[p]) = (F_out(x[p]), F_act(x[p])), qt = mean(act).
With b1 == 0 the first LayerNorm collapses analytically:

    LN1(x*w1) = tau * (w1-mean(w1))/std(w1) * g1 + be1,
    tau = sw*x / (sw*|x| + EPS),  sw = std(w1, ddof=1),  tau in [-1, 1].

So F = G(tau).  G has eps-kinks at tau=0 from the inner LayerNorms, but the
tau-map sends all |x| >= ~2e-3 to |tau| >= 0.95: 99.8% of elements live in
|tau| in [0.95, 1] where G is glass-smooth.  We fit even/odd parts of G_out
and of sigmoid(s_a) there as degree-2 polynomials in the recentered
zeta = (tau^2-c0)/hw (~5e-7 max err), evaluate on-device, and exactly patch
the ~1700 elements with |x| < x_cut on the host (float64 reference math).

Per-core device program (raw bass, no Tile; shard = [128, 977], zero-pad):
  SYNC : dma x_c in | dma yo_c out | dma qsum | final sem reset
  ACT  : Abs -> Ln(.+eps) -> Exp(-.) == r = 1/(sw|x|+eps);
         final x_out affine (Copy scale/bias) -- one table set total
  DVE  : tau=(x*sw)*r ; zeta ; fused x_out Horner (top coeffs deferred into
         the combine scalar and the ACT affine)
  GPSIM: eps memset; the act-function Horner with fused accum_out partial
         sums (its deferred scale/constant are applied to the sum on host)
Engines self-serialize internally (ACT ops don't overlap; DVE drains per
op), so only cross-engine edges need sems: standalone wait_ge instructions.
qt = (oa2 * device_sum + ea0*N - pad - patch corrections) / 1e6 on host.
"""

import contextlib

import numpy as np

H = 20
EPS = 1e-5
P_TOTAL = 1_000_000
N_CORES = 8
SHARD = P_TOTAL // N_CORES          # 125000
FREE = 978                          # 128*978 = 125184 >= SHARD
NPAD = 128 * FREE - SHARD           # 184 zero-pad elements per core

TAU_STAR = 0.95
FIT_TOL = 5e-6
WIDTHS = [200, 600, 178]            # uneven: short lead-in, fast tail drain
NCH = len(WIDTHS)
OFFS = [128 * sum(WIDTHS[:i]) for i in range(NCH)]      # dram elem offsets
OUT_WIDTHS = WIDTHS[:-1] + [WIDTHS[-1] + 1]             # +1 col: qt sums
OUT_OFFS = [128 * sum(OUT_WIDTHS[:i]) for i in range(NCH)]

_CACHE = {}


# ----------------------------------------------------------------- host math
def _sigmoid(z):
    return 1.0 / (1.0 + np.exp(-np.clip(z, -60, 60)))


def _ln1d(v, g, b):
    m = v.mean(axis=-1, keepdims=True)
    s = v.std(axis=-1, ddof=1, keepdims=True)
    return (v - m) / (s + EPS) * g + b


def _cell(x, wi, gi, bi, bh, gc, bc):
    # hx = cx = 0: LN(hx@wh.T) == bh (constant); forget gate multiplies cx==0
    gates = _ln1d(x @ wi.T, gi, bi) + bh
    i, _f, o, g = np.split(gates, 4, axis=-1)
    c = _sigmoid(i) * np.tanh(g)
    return _sigmoid(o) * np.tanh(_ln1d(c, gc, bc))


def _net_of_tau(tau, d):
    """Exact (G_out, s_a) in float64 as functions of tau (includes all eps)."""
    w1 = d['w1'].ravel()
    A = (w1 - w1.mean()) / w1.std(ddof=1)
    xt = np.tanh(tau[:, None] * (A * d['g1'])[None, :] + d['be1'][None, :])
    for l in (0, 1):
        xt = _cell(xt, d[f'wi{l}'], d[f'gi{l}'], d[f'bi{l}'],
                   d[f'bh{l}'], d[f'gc{l}'], d[f'bc{l}'])
    out = xt @ d['wo'].T + d['bo']
    sa = xt @ d['wa'].T + d['ba']
    return out.ravel(), sa.ravel()


def _fit2(zeta, target):
    c = np.polynomial.chebyshev.chebfit(zeta, target, 2)
    p = np.polynomial.chebyshev.cheb2poly(c)
    err = np.abs(np.polynomial.polynomial.polyval(zeta, p) - target).max()
    assert err < FIT_TOL, f"degree-2 fit err {err:.2e} exceeds {FIT_TOL}"
    assert abs(p[2]) > 1e-12
    return p


class _Fit:
    """Degree-2 even/odd model on |tau| in [TAU_STAR, 1]:
    G = E(zeta) + tau*O(zeta), zeta = (tau^2-c0)/hw."""

    def __init__(self, d):
        self.sw = d['w1'].ravel().std(ddof=1)
        self.c0 = (TAU_STAR ** 2 + 1.0) / 2.0
        self.hw = (1.0 - TAU_STAR ** 2) / 2.0
        t = np.sqrt(np.linspace(TAU_STAR ** 2, 1.0, 20001))
        gp_out, gp_sa = _net_of_tau(t, d)
        gm_out, gm_sa = _net_of_tau(-t, d)
        gp_act, gm_act = _sigmoid(gp_sa), _sigmoid(gm_sa)
        zeta = (t * t - self.c0) / self.hw
        self.eo = _fit2(zeta, (gp_out + gm_out) / 2)           # even, out
        self.oo = _fit2(zeta, (gp_out - gm_out) / 2 / t)       # odd, out
        self.ea = _fit2(zeta, (gp_act + gm_act) / 2)           # even, act
        self.oa = _fit2(zeta, (gp_act - gm_act) / 2 / t)       # odd, act
        # device evaluates in the uncentered v = tau^2/hw = zeta + c0/hw:
        # compose each poly with zeta = v - c0/hw (top coeff is invariant;
        # the deferred small top-coefficient scales absolute rounding well
        # below fit error)
        z0 = -self.c0 / self.hw
        P = np.polynomial.Polynomial
        shift = P([z0, 1.0])
        self.eo_d = P(self.eo)(shift).coef
        self.oo_d = P(self.oo)(shift).coef
        self.ea_d = P(self.ea)(shift).coef
        self.oa_d = P(self.oa)(shift).coef

    def tau_of(self, x):
        return self.sw * x / (np.abs(x) * self.sw + EPS)

    def eval_poly(self, tau):
        """Float64 replica of the device computation (for corrections)."""
        P = np.polynomial.polynomial.polyval
        zeta = (tau * tau - self.c0) / self.hw
        out = P(zeta, self.eo) + tau * P(zeta, self.oo)
        act = P(zeta, self.ea) + tau * P(zeta, self.oa)
        return out, act


# -------------------------------------------------------- raw bass builder
def _build(fit):
    import concourse.bass as bass
    import concourse.mybir as mybir

    f32 = mybir.dt.float32
    AF = mybir.ActivationFunctionType
    OP = mybir.AluOpType

    from unittest import mock

    # no Tile: orderings are carried by explicit semaphores, so the heavy
    # full-drain barriers (init + Block-end, ~13us of EVENT_SEMAPHORE churn
    # and engine drains per execution) are replaced by the cheap
    # sequencer-level sem-only barrier, which still terminates every
    # engine's stream.
    # drop both framework barriers (init + Block-end): all orderings are
    # carried by explicit semaphores; the sync stream's final s_out wait
    # transitively proves every engine done before the sem_clears
    barrier_patch = mock.patch.object(bass.Bass, "all_engine_barrier",
                                      lambda self, **kw: None)
    barrier_patch.start()
    try:
        nc = _build_inner(bass, mybir, f32, AF, OP, fit)
    finally:
        barrier_patch.stop()
    return nc


def _build_inner(bass, mybir, f32, AF, OP, fit):
    nc = bass.Bass()
    # chunk-contiguous flat layout: one dense transfer per chunk; the last
    # chunk carries one extra column holding the qt partial-sum vector (a
    # separate [128,1] DMA would cost ~6.5us in 4-byte descriptors)
    x_d = nc.dram_tensor("x", [128 * FREE], f32, kind="ExternalInput")
    out_d = nc.dram_tensor("out", [128 * FREE + 128], f32,
                           kind="ExternalOutput")

    def in_ap(c):
        return bass.AP(x_d, OFFS[c], [[WIDTHS[c], 128], [1, WIDTHS[c]]])

    def out_ap(c):
        w = OUT_WIDTHS[c]
        return bass.AP(out_d, OUT_OFFS[c], [[w, 128], [1, w]])

    sw = float(fit.sw)
    inv_hw = float(1.0 / fit.hw)
    zoff = float(-fit.c0 / fit.hw)
    eo, oo, ea, oa = fit.eo, fit.oo, fit.ea, fit.oa
    widths = WIDTHS

    ctx = contextlib.ExitStack()
    sem = lambda name: ctx.enter_context(nc.semaphore(name))
    sb = lambda name, W: ctx.enter_context(nc.sbuf_tensor(name, [128, W], f32))

    with ctx:
        s_eps = sem("s_eps")
        s_in = sem("s_in")
        s_r = sem("s_r")
        s_zeta = sem("s_zeta")
        s_comb = sem("s_comb")
        s_gq = sem("s_gq")
        s_yo = sem("s_yo")
        s_q = sem("s_q")
        s_out = sem("s_out")
        all_sems = [s_eps, s_in, s_r, s_zeta, s_comb, s_gq, s_yo, s_q, s_out]

        eps_t = sb("eps_t", 1)
        scratch = sb("scratch", 1)
        qparts = sb("qparts", NCH)
        qred = sb("qred", 1)
        X, T, R, TAU, ZETA = [], [], [], [], []
        WE, WO, VE, VO, YP, YO = [], [], [], [], [], []
        for c, W in enumerate(widths):
            X.append(sb(f"x{c}", W)); T.append(sb(f"t{c}", W))
            R.append(sb(f"r{c}", W)); TAU.append(sb(f"tau{c}", W))
            ZETA.append(sb(f"zeta{c}", W))
            WE.append(sb(f"wE{c}", W)); WO.append(sb(f"wO{c}", W))
            VE.append(sb(f"vE{c}", W)); VO.append(sb(f"vO{c}", W))
            YP.append(sb(f"yp{c}", W))
            YO.append(sb(f"yo{c}", W + (1 if c == NCH - 1 else 0)))
        qdump = sb("qdump", max(widths))

        with nc.Block() as block:

            @block.sync
            def _(sync):
                for c in range(NCH):
                    sync.dma_start(X[c][:, :], in_ap(c)).then_inc(s_in, 16)
                for c in range(NCH):
                    sync.wait_ge(s_yo, c + 1)
                    if c == NCH - 1:
                        sync.wait_ge(s_q, 1)
                    sync.dma_start(out_ap(c), YO[c][:, :]).then_inc(s_out, 16)
                sync.wait_ge(s_out, 16 * NCH)
                for s in all_sems:
                    sync.sem_clear(s)

            @block.scalar
            def _(scalar):
                # dummy op before any wait: anchors the one ACT_TABLE_LOAD
                # (ln/exp set) at t~0, overlapped with the input DMA
                nc.scalar.activation(scratch[:, 0:1], eps_t[:, 0:1], AF.Ln)
                scalar.wait_ge(s_eps, 1)
                for c in range(NCH):
                    # r = exp(-ln(sw|x| + eps)) = 1/(sw|x|+eps)
                    scalar.wait_ge(s_in, 16 * (c + 1))
                    nc.scalar.activation(T[c][:, :], X[c][:, :], AF.Abs,
                                         scale=sw)
                    nc.scalar.activation(R[c][:, :], T[c][:, :], AF.Ln,
                                         bias=eps_t[:, 0:1])
                    nc.scalar.activation(R[c][:, :], R[c][:, :], AF.Exp,
                                         scale=-1.0).then_inc(s_r, 1)
                for c in range(NCH):
                    # x_out = oo2 * yp + eo0
                    scalar.wait_ge(s_comb, c + 1)
                    nc.scalar.activation(YO[c][:, :widths[c]], YP[c][:, :],
                                         AF.Copy, scale=float(oo[2]),
                                         bias=float(eo[0])).then_inc(s_yo, 1)

            @block.vector
            def _(vector):
                stt = nc.vector.scalar_tensor_tensor
                ts = nc.vector.tensor_scalar
                for c in range(NCH):
                    Z = ZETA[c][:, :]
                    vector.wait_ge(s_r, c + 1)
                    stt(TAU[c][:, :], X[c][:, :], sw, R[c][:, :],
                        op0=OP.mult, op1=OP.mult)
                    stt(Z, TAU[c][:, :], inv_hw, TAU[c][:, :],
                        op0=OP.mult, op1=OP.mult)
                    ts(Z, Z, zoff, None, OP.add).then_inc(s_zeta, 1)
                    # x_out chain: yp = (eo2/oo2)*wE + wO, wE=(z+eo1/eo2)z,
                    # wO = ((z+oo1/oo2)z + oo0/oo2)*tau
                    stt(WE[c][:, :], Z, float(eo[1] / eo[2]), Z,
                        op0=OP.add, op1=OP.mult)
                    stt(WO[c][:, :], Z, float(oo[1] / oo[2]), Z,
                        op0=OP.add, op1=OP.mult)
                    stt(WO[c][:, :], WO[c][:, :], float(oo[0] / oo[2]),
                        TAU[c][:, :], op0=OP.add, op1=OP.mult)
                    stt(YP[c][:, :], WE[c][:, :], float(eo[2] / oo[2]),
                        WO[c][:, :], op0=OP.mult, op1=OP.add
                        ).then_inc(s_comb, 1)
                    _emit_act_chain(nc.vector, nc, c, widths, ZETA, TAU,
                                    VE, VO, qdump, qparts, ea, oa, OP,
                                    s_gq)
                vector.wait_ge(s_gq, NCH)
                W_last = WIDTHS[-1]
                nc.vector.tensor_reduce(
                    YO[NCH - 1][:, W_last:W_last + 1], qparts[:, :],
                    axis=mybir.AxisListType.X, op=OP.add).then_inc(s_q, 1)

            @block.gpsimd
            def _(gpsimd):
                gpsimd.memset(eps_t[:, :], float(EPS)).then_inc(s_eps, 1)

            @block.tensor
            def _(tensor):
                tensor.nop()

    nc.finalize()
    return nc


def _emit_act_chain(eng, nc, c, widths, ZETA, TAU, VE, VO, qdump, qparts,
                    ea, oa, OP, s_gq):
    """act/oa2 (sans ea0 term): vE=(z+ea1/ea2)z; vO=((z+oa1/oa2)z+oa0/oa2)*tau;
    v = (ea2/oa2)*vE + vO, partial sums via accum_out."""
    Z = ZETA[c][:, :]
    stt = eng.scalar_tensor_tensor
    stt(VE[c][:, :], Z, float(ea[1] / ea[2]), Z, op0=OP.add, op1=OP.mult)
    stt(VO[c][:, :], Z, float(oa[1] / oa[2]), Z, op0=OP.add, op1=OP.mult)
    stt(VO[c][:, :], VO[c][:, :], float(oa[0] / oa[2]), TAU[c][:, :],
        op0=OP.add, op1=OP.mult)
    stt(qdump[:, :widths[c]], VE[c][:, :], float(ea[2] / oa[2]),
        VO[c][:, :], op0=OP.mult, op1=OP.add,
        accum_out=qparts[:, c:c + 1]).then_inc(s_gq, 1)


def _get_compiled(inputs):
    d = {k: np.asarray(v, dtype=np.float64) for k, v in inputs.items()
         if k != 'x'}
    key = hash(tuple(sorted((k, v.tobytes()) for k, v in d.items())))
    if key in _CACHE:
        return _CACHE[key]
    assert np.abs(d['b1']).max() == 0.0, "b1 != 0 breaks the LN1 reduction"
    fit = _Fit(d)
    nc = _build(fit)
    entry = (nc, fit, d)
    _CACHE[key] = entry
    return entry


def make_in_maps(x):
    shards = np.asarray(x, np.float32).ravel().reshape(N_CORES, SHARD)
    in_maps = []
    for i in range(N_CORES):
        xp = np.zeros(128 * FREE, dtype=np.float32)
        xp[:SHARD] = shards[i]
        in_maps.append({"x": xp})
    return in_maps


def postprocess(results, x, fit, d):
    x = np.asarray(x, np.float64).ravel()
    per_core, vsum = [], 0.0
    for r in results:
        flat = np.asarray(r["out"], np.float32).ravel()
        segs = []
        for c in range(NCH):
            blk = flat[OUT_OFFS[c]:OUT_OFFS[c] + 128 * OUT_WIDTHS[c]]
            blk = blk.reshape(128, OUT_WIDTHS[c])
            segs.append(blk[:, :WIDTHS[c]].ravel())
            if c == NCH - 1:
                # qt partial-sum vector rides in the extra output column;
                # sums of (act - ea0)/oa2 over ALL 128*FREE elements
                vsum += float(blk[:, WIDTHS[c]].astype(np.float64).sum())
        per_core.append(np.concatenate(segs)[:SHARD])
    x_out = np.concatenate(per_core).astype(np.float64)
    qtot = float(fit.oa[2]) * vsum + float(fit.ea[0]) * (N_CORES * 128 * FREE)
    # pad elements (x=0 -> tau=0) contribute the device act value at tau=0
    _, act0 = fit.eval_poly(np.zeros(1))
    qtot -= N_CORES * NPAD * float(act0[0])
    # exact host patch for |x| below the fit domain
    x_cut = TAU_STAR * EPS / (fit.sw * (1.0 - TAU_STAR))
    idx = np.where(np.abs(x) < x_cut)[0]
    if idx.size:
        tau_p = fit.tau_of(x[idx])
        out_dev, act_dev = fit.eval_poly(tau_p)       # what the device wrote
        out_ex, sa_ex = _net_of_tau(tau_p, d)
        x_out[idx] = out_ex
        qtot += float((_sigmoid(sa_ex) - act_dev).sum())
    qt = np.array([qtot / P_TOTAL], dtype=np.float32)
    return x_out.reshape(P_TOTAL, 1).astype(np.float32), qt


# ------------------------------------------------------------------- kernel
def kernel(**inputs):
    from concourse.bass_utils import run_bass_kernel_spmd

    nc, fit, d = _get_compiled(inputs)
    x = np.asarray(inputs['x'], dtype=np.float32).ravel()
    assert x.shape == (P_TOTAL,)
    res = run_bass_kernel_spmd(nc, make_in_maps(x),
                               core_ids=list(range(N_CORES)))
    return postprocess(res.results, x, fit, d)
